# revision 13
# baseline (speedup 1.0000x reference)
"""Distributed attention kernel for 8 TRN2 NeuronCores (v3).

Sharding: core c -> (batch b = c//2, head-half hh = c%2).  Each core computes
LN(x_b) for all 2048 rows, q for its 8 heads, k/v over the first KEYSC-1
rows of a HOST-PERMUTED x (visible rows first, masked rows after; the output
is un-permuted on the host), l2norm cosine attention with the null k/v
appended at slot KEYSC-1, and a partial out @ wo[head-slice].  Host sums the
two partial outputs per batch.

v3 notes: the PE HAM clock gate re-throttles to 1.2 GHz after any ~3.4us PE
idle window, so every phase boundary gap matters.  The l2-norm stage shares
the projection-phase PSUM pool (so its matmuls interleave with the tail of
the projections), the softmax division runs per-m inside the attention loop,
and all 1/x go through the single-op reciprocal_approx_fast.
"""

import sys

sys.path.insert(0, "/opt/trn_rl_repo")

import numpy as np  # noqa: E402
import ml_dtypes  # noqa: E402

import concourse.bacc as bacc  # noqa: E402
import concourse.bass as bass  # noqa: E402
import concourse.tile as tile  # noqa: E402
from concourse import mybir  # noqa: E402
from concourse.bass_utils import run_bass_kernel_spmd  # noqa: E402

BF = ml_dtypes.bfloat16
F32 = mybir.dt.float32
BF16 = mybir.dt.bfloat16
AF = mybir.ActivationFunctionType
MUL = mybir.AluOpType.mult

P = 128
N = 2048          # query rows per batch
D = 1024          # model dim
HC = 8            # heads per core
IC = 512          # inner dim per core
NEG = -1.0e4
EPS_LN = 1e-5
EPS_L2 = 1e-12
SCALE = 8.0

KEYSC = 1152      # key slots: [0:nvis) visible, pads, null at KEYSC-1


def _chunks(total, step=512):
    return [(c, min(c + step, total)) for c in range(0, total, step)]


def build_nc(keysc=KEYSC):
    kcn = keysc // P
    nc = bacc.Bacc(None, target_bir_lowering=False)

    xT_d = nc.dram_tensor("xT", [D, N], BF16, kind="ExternalInput")
    wq_d = nc.dram_tensor("wq", [D, IC], BF16, kind="ExternalInput")
    wk_d = nc.dram_tensor("wk", [D, IC], BF16, kind="ExternalInput")
    wv_d = nc.dram_tensor("wv", [D, IC], BF16, kind="ExternalInput")
    wo_d = nc.dram_tensor("wo", [IC, D], BF16, kind="ExternalInput")
    nullk_d = nc.dram_tensor("nullk", [P, 4], BF16, kind="ExternalInput")
    nullv_d = nc.dram_tensor("nullv", [1, HC * 65], BF16, kind="ExternalInput")
    mask_d = nc.dram_tensor("maskcol", [P, kcn], F32, kind="ExternalInput")
    qks_d = nc.dram_tensor("qks", [P, 1], F32, kind="ExternalInput")
    out_d = nc.dram_tensor("out", [N, D], F32, kind="ExternalOutput")

    with tile.TileContext(nc) as tc:
        with (
            tc.tile_pool(name="consts", bufs=1) as cns,
            tc.tile_pool(name="qkv", bufs=1) as qkv,
            tc.tile_pool(name="wop", bufs=1) as wop,
            tc.tile_pool(name="rep", bufs=4) as repp,
            tc.tile_pool(name="dram", bufs=1, space="DRAM") as drp,
        ):
            qT = qkv.tile([P, 4, N], BF16)
            kT = qkv.tile([P, 4, keysc], BF16)
            v_sb = qkv.tile([P, kcn, HC * 65], BF16)
            oT = qkv.tile([P, 4, N], BF16)

            bnc_dr = drp.tile([2, N], BF16, name="bnc")
            den_dr = drp.tile([8, N], BF16, name="den")
            rec_dr = drp.tile([8, N], BF16, name="rec")

            # ---------- phase A+B: LN, projections, l2 norms ----------
            with (
                tc.tile_pool(name="xp", bufs=1) as xp,
                tc.tile_pool(name="wp", bufs=1) as wp,
                tc.tile_pool(name="asml", bufs=2) as sml,
                tc.tile_pool(name="ascr", bufs=2) as scr,
                tc.tile_pool(name="bsml", bufs=2) as bml,
                tc.tile_pool(name="lnps", bufs=2, space="PSUM") as lnps,
                tc.tile_pool(name="pjps", bufs=3, space="PSUM") as pjps,
            ):
                xT = xp.tile([P, 8, N], BF16)
                xre = xT_d.rearrange("(f p) r -> f p r", p=P)
                # first x chunk ahead of everything else in the DMA queues
                for f in range(8):
                    nc.sync.dma_start(out=xT[:, f, 0:512],
                                      in_=xre[f, :, 0:512])
                wq_sb = wp.tile([P, 8, IC], BF16, tag="wq")
                wk_sb = wp.tile([P, 8, IC], BF16, tag="wk")
                wv_sb = wp.tile([P, 8, IC], BF16, tag="wv")
                nc.sync.dma_start(
                    out=wq_sb, in_=wq_d.rearrange("(f p) j -> p f j", p=P))
                nc.sync.dma_start(
                    out=wk_sb, in_=wk_d.rearrange("(f p) j -> p f j", p=P))
                nc.sync.dma_start(
                    out=wv_sb, in_=wv_d.rearrange("(f p) j -> p f j", p=P))

                ones1b = cns.tile([P, 1], BF16)
                nc.vector.memset(ones1b, 1.0)
                blkdiag = cns.tile([P, 2], BF16)
                nc.vector.memset(blkdiag, 0.0)
                nc.vector.memset(blkdiag[0:64, 0:1], 1.0)
                nc.vector.memset(blkdiag[64:128, 1:2], 1.0)
                maskc = cns.tile([P, kcn], F32)
                nc.sync.dma_start(out=maskc, in_=mask_d[:, :])
                nullk_sb = cns.tile([P, 4], BF16)
                nc.sync.dma_start(out=nullk_sb, in_=nullk_d[:, :])
                qks_sb = cns.tile([P, 1], F32)
                nc.sync.dma_start(out=qks_sb, in_=qks_d[:, :])
                eps_col = cns.tile([P, 1], F32)
                nc.vector.memset(eps_col, EPS_LN)
                sc_col = cns.tile([P, 1], F32)
                nc.vector.memset(sc_col, 1.0 / float(D * D))
                eps2_col = cns.tile([P, 1], F32)
                nc.vector.memset(eps2_col, EPS_L2 * EPS_L2)
                wo_sb = wop.tile([P, 4, D], BF16)
                nc.sync.dma_start(
                    out=wo_sb, in_=wo_d.rearrange("(m p) j -> p m j", p=P))
                nc.vector.memset(
                    v_sb.rearrange(
                        "p t (h c) -> p t h c", c=65)[:, :, :, 64:65],
                    1.0)

                for ci, (c0, c1) in enumerate(_chunks(N)):
                    w = c1 - c0
                    if ci > 0:
                        for f in range(8):
                            nc.sync.dma_start(out=xT[:, f, c0:c1],
                                              in_=xre[f, :, c0:c1])
                    sA = lnps.tile([1, 512], F32, tag="lA", name="sA")
                    sB = lnps.tile([1, 512], F32, tag="lB", name="sB")
                    for f in range(8):
                        xc = xT[:, f, c0:c1]
                        sq = scr.tile([P, 512], BF16, tag="sq", name="sq")
                        nc.vector.tensor_mul(sq[:, 0:w], xc, xc)
                        nc.tensor.matmul(sA[:, 0:w], ones1b, xc,
                                         start=(f == 0), stop=(f == 7))
                        nc.tensor.matmul(sB[:, 0:w], ones1b, sq[:, 0:w],
                                         start=(f == 0), stop=(f == 7))
                    # s = 1/sqrt(var+eps); ms = mean*s  (rows bf16)
                    sumr = sml.tile([1, 512], F32, tag="sumr", name="sumr")
                    nc.vector.tensor_copy(sumr[:, 0:w], sA[:, 0:w])
                    a1 = sml.tile([1, 512], F32, tag="a1", name="a1")
                    nc.vector.tensor_mul(a1[:, 0:w], sumr[:, 0:w],
                                         sumr[:, 0:w])
                    t1 = sml.tile([1, 512], F32, tag="t1", name="t1")
                    nc.vector.tensor_scalar(t1[:, 0:w], sB[:, 0:w],
                                            float(D), None, MUL)
                    nc.vector.tensor_sub(t1[:, 0:w], t1[:, 0:w], a1[:, 0:w])
                    sd = sml.tile([1, 512], F32, tag="sd", name="sd")
                    nc.scalar.activation(sd[:, 0:w], t1[:, 0:w], AF.Sqrt,
                                         scale=sc_col[0:1, :],
                                         bias=eps_col[0:1, :])
                    sf = sml.tile([1, 512], F32, tag="sf", name="sf")
                    nc.vector.reciprocal_approx_fast(sf[:, 0:w], sd[:, 0:w])
                    row_s = sml.tile([1, 512], BF16, tag="rows", name="row_s")
                    nc.vector.tensor_copy(row_s[:, 0:w], sf[:, 0:w])
                    nc.vector.tensor_mul(a1[:, 0:w], sumr[:, 0:w],
                                         sf[:, 0:w])
                    row_m = sml.tile([1, 512], BF16, tag="rowm", name="row_m")
                    nc.vector.tensor_scalar(row_m[:, 0:w], a1[:, 0:w],
                                            1.0 / float(D), None, MUL)
                    nc.sync.dma_start(out=bnc_dr[0:1, c0:c1],
                                      in_=row_s[:, 0:w])
                    nc.sync.dma_start(out=bnc_dr[1:2, c0:c1],
                                      in_=row_m[:, 0:w])
                    rep_s = repp.tile([P, 512], BF16, tag="rep", name="rep_s")
                    rep_m = repp.tile([P, 512], BF16, tag="rep", name="rep_m")
                    for (i, r) in ((0, rep_s), (1, rep_m)):
                        src = bnc_dr[i, c0:c1]
                        nc.sync.dma_start(
                            out=r[:, 0:w],
                            in_=bass.AP(tensor=src.tensor, offset=src.offset,
                                        ap=[[0, P]] + src.ap))
                    for f in range(8):
                        nc.vector.tensor_mul(xT[:, f, c0:c1], xT[:, f, c0:c1],
                                             rep_s[:, 0:w])
                        nc.vector.tensor_sub(xT[:, f, c0:c1], xT[:, f, c0:c1],
                                             rep_m[:, 0:w])
                    # q projection for this chunk
                    for m in range(4):
                        qp = pjps.tile([P, 512], F32, tag="pj", name="qp")
                        for f in range(8):
                            nc.tensor.matmul(
                                qp[:, 0:w], wq_sb[:, f, m * P:(m + 1) * P],
                                xT[:, f, c0:c1],
                                start=(f == 0), stop=(f == 7))
                        nc.scalar.copy(qT[:, m, c0:c1], qp[:, 0:w])
                    # k projection (only columns < keysc)
                    k1 = min(c1, keysc)
                    if c0 < keysc:
                        kw = k1 - c0
                        for m in range(4):
                            kp = pjps.tile([P, 512], F32, tag="pj", name="kp")
                            for f in range(8):
                                nc.tensor.matmul(
                                    kp[:, 0:kw],
                                    wk_sb[:, f, m * P:(m + 1) * P],
                                    xT[:, f, c0:k1],
                                    start=(f == 0), stop=(f == 7))
                            nc.scalar.copy(kT[:, m, c0:k1], kp[:, 0:kw])
                        # v projection: 128-row tiles within this chunk
                        for rt in range(c0 // P, k1 // P):
                            vp = pjps.tile([P, 512], F32, tag="pj", name="vp")
                            for f in range(8):
                                nc.tensor.matmul(
                                    vp, xT[:, f, rt * P:(rt + 1) * P],
                                    wv_sb[:, f, :],
                                    start=(f == 0), stop=(f == 7))
                            nc.scalar.copy(
                                v_sb[:, rt, :].rearrange(
                                    "p (h c) -> p h c", c=65)[:, :, 0:64],
                                vp.rearrange("p (h c) -> p h c", c=64))
                    if ci == 2:
                        # k/v fully projected: null k/v overwrite (keysc-1)
                        nc.sync.dma_start(
                            out=kT[:, :, keysc - 1:keysc],
                            in_=nullk_sb.rearrange("p (m o) -> p m o", o=1))
                        nc.sync.dma_start(out=v_sb[127:128, kcn - 1, :],
                                          in_=nullv_d[:, :])
                        # l2 of k for all m can start during chunk 3's work
                        for m in range(4):
                            sqk = scr.tile([P, keysc], BF16, tag="sqq",
                                           bufs=2, name="sqk")
                            nc.vector.tensor_mul(sqk, kT[:, m, :],
                                                 kT[:, m, :])
                            nstk = bml.tile([2, keysc], F32, tag="l2a",
                                            name="nstk")
                            for (d0, d1) in _chunks(keysc):
                                t2 = pjps.tile([2, 512], F32, tag="pj",
                                               name="t2k")
                                nc.tensor.matmul(t2[:, 0:d1 - d0], blkdiag,
                                                 sqk[:, d0:d1],
                                                 start=True, stop=True)
                                nc.vector.tensor_copy(nstk[:, d0:d1],
                                                      t2[:, 0:d1 - d0])
                            sdk = bml.tile([2, keysc], F32, tag="l2a",
                                           name="sdk")
                            nc.scalar.activation(sdk, nstk, AF.Sqrt,
                                                 bias=eps2_col[0:2, :])
                            nc.vector.reciprocal_approx_fast(nstk, sdk)
                            nbk = bml.tile([2, keysc], BF16, tag="l2b",
                                           name="nbk")
                            nc.vector.tensor_scalar(nbk, nstk, SCALE,
                                                    None, MUL)
                            bk = drp.tile([2, keysc], BF16, tag="bk", bufs=4,
                                          name=f"bk{m}")
                            nc.sync.dma_start(out=bk, in_=nbk)
                            repk = repp.tile([P, keysc], BF16, tag="rpq",
                                             bufs=2, name="repk")
                            for h2 in range(2):
                                src = bk[h2, :]
                                nc.sync.dma_start(
                                    out=repk[64 * h2:64 * (h2 + 1), :],
                                    in_=bass.AP(tensor=src.tensor,
                                                offset=src.offset,
                                                ap=[[0, 64]] + src.ap))
                            nc.vector.tensor_mul(kT[:, m, :], kT[:, m, :],
                                                 repk)
                # l2 of q per m (q complete after last chunk)
                for m in range(4):
                    sqq = scr.tile([P, N], BF16, tag="sqq", bufs=2,
                                   name="sqq")
                    nc.vector.tensor_mul(sqq, qT[:, m, :], qT[:, m, :])
                    nst = bml.tile([2, N], F32, tag="l2a", name="nstq")
                    for (d0, d1) in _chunks(N):
                        t2 = pjps.tile([2, 512], F32, tag="pj", name="t2")
                        nc.tensor.matmul(t2[:, 0:d1 - d0], blkdiag,
                                         sqq[:, d0:d1], start=True, stop=True)
                        nc.vector.tensor_copy(nst[:, d0:d1],
                                              t2[:, 0:d1 - d0])
                    sd2 = bml.tile([2, N], F32, tag="l2a", name="sd2")
                    nc.scalar.activation(sd2, nst, AF.Sqrt,
                                         bias=eps2_col[0:2, :])
                    nc.vector.reciprocal_approx_fast(nst, sd2)
                    nbf = bml.tile([2, N], BF16, tag="l2b", name="nbfq")
                    nc.vector.tensor_copy(nbf, nst)
                    bq = drp.tile([2, N], BF16, tag="bq", bufs=4,
                                  name=f"bq{m}")
                    nc.sync.dma_start(out=bq, in_=nbf)
                    repq = repp.tile([P, N], BF16, tag="rpq", bufs=2,
                                     name="repq")
                    for h2 in range(2):
                        src = bq[h2, :]
                        nc.sync.dma_start(
                            out=repq[64 * h2:64 * (h2 + 1), :],
                            in_=bass.AP(tensor=src.tensor, offset=src.offset,
                                        ap=[[0, 64]] + src.ap))
                    nc.vector.tensor_scalar(repq, repq, qks_sb[:, 0:1],
                                            None, MUL)
                    nc.vector.tensor_mul(qT[:, m, :], qT[:, m, :], repq)

            # ---------- phase C: attention + per-m softmax division --------
            with (
                tc.tile_pool(name="accp", bufs=2, space="PSUM") as accp,
                tc.tile_pool(name="simp", bufs=2, space="PSUM") as simp,
                tc.tile_pool(name="expp", bufs=3) as expp,
                tc.tile_pool(name="omp", bufs=3) as omp,
                tc.tile_pool(name="dsml", bufs=2) as dsml,
            ):
                for m in range(4):
                    for rc in range(2):
                        ops = [accp.tile([65, 1024], F32, tag="acc",
                                         name=f"av{i}") for i in range(2)]
                        for kc in range(kcn):
                            for h2 in range(2):
                                sim = simp.tile([P, 1024], F32, tag="sim",
                                                name="sim")
                                for nh in range(2):
                                    r0 = rc * 1024 + nh * 512
                                    nc.tensor.matmul(
                                        sim[:, nh * 512:(nh + 1) * 512],
                                        kT[64 * h2:64 * (h2 + 1),
                                           m, kc * P:(kc + 1) * P],
                                        qT[64 * h2:64 * (h2 + 1),
                                           m, r0:r0 + 512],
                                        start=True, stop=True)
                                e = expp.tile([P, 1024], BF16, tag="e",
                                              name="e")
                                nc.scalar.activation(
                                    e, sim, AF.Exp,
                                    bias=maskc[:, kc:kc + 1])
                                for nh in range(2):
                                    nc.tensor.matmul(
                                        ops[h2][:, nh * 512:(nh + 1) * 512],
                                        v_sb[:, kc,
                                             (2 * m + h2) * 65:
                                             (2 * m + h2 + 1) * 65],
                                        e[:, nh * 512:(nh + 1) * 512],
                                        start=(kc == 0), stop=(kc == kcn - 1))
                        for h2 in range(2):
                            om = omp.tile([65, 1024], BF16, tag="om",
                                          name="om")
                            with nc.allow_low_precision("bf16 numer/denom"):
                                nc.vector.tensor_copy(om, ops[h2])
                            nc.sync.dma_start(
                                out=oT[64 * h2:64 * (h2 + 1), m,
                                       rc * 1024:(rc + 1) * 1024],
                                in_=om[0:64, :])
                            nc.sync.dma_start(
                                out=den_dr[2 * m + h2:2 * m + h2 + 1,
                                           rc * 1024:(rc + 1) * 1024],
                                in_=om[64:65, :])
                    # divide this m's numerators while m+1 attends
                    dsb = dsml.tile([2, N], BF16, tag="dsb", name="dsb")
                    nc.sync.dma_start(out=dsb, in_=den_dr[2 * m:2 * m + 2, :])
                    dff = dsml.tile([2, N], F32, tag="dff", name="dff")
                    nc.vector.tensor_copy(dff, dsb)
                    drf = dsml.tile([2, N], F32, tag="dff", name="drf")
                    nc.vector.reciprocal_approx_fast(drf, dff)
                    drb = dsml.tile([2, N], BF16, tag="dsb", name="drb")
                    with nc.allow_low_precision("bf16 recip"):
                        nc.vector.tensor_copy(drb, drf)
                    nc.sync.dma_start(out=rec_dr[2 * m:2 * m + 2, :], in_=drb)
                    repd = repp.tile([P, N], BF16, tag="rpq", bufs=2,
                                     name="repd")
                    for h2 in range(2):
                        src = rec_dr[2 * m + h2, :]
                        nc.sync.dma_start(
                            out=repd[64 * h2:64 * (h2 + 1), :],
                            in_=bass.AP(tensor=src.tensor, offset=src.offset,
                                        ap=[[0, 64]] + src.ap))
                    nc.vector.tensor_mul(oT[:, m, :], oT[:, m, :], repd)

            # ---------- phase D: output projection ----------
            with (
                tc.tile_pool(name="dps", bufs=4, space="PSUM") as dps,
                tc.tile_pool(name="dscr", bufs=3) as scr2,
            ):
                for rt in range(16):
                    for n2 in range(2):
                        op = dps.tile([P, 512], F32, tag="op", name="op")
                        for m in range(4):
                            nc.tensor.matmul(
                                op, oT[:, m, rt * P:(rt + 1) * P],
                                wo_sb[:, m, n2 * 512:(n2 + 1) * 512],
                                start=(m == 0), stop=(m == 3))
                        sg = scr2.tile([P, 512], F32, tag="sg", name="sg")
                        if n2 == 0:
                            nc.scalar.copy(sg, op)
                        else:
                            nc.vector.tensor_copy(sg, op)
                        nc.sync.dma_start(
                            out=out_d[rt * P:(rt + 1) * P,
                                      n2 * 512:(n2 + 1) * 512],
                            in_=sg)

    nc.finalize()
    return nc


_NC = {}


def _get_nc(keysc=KEYSC):
    if keysc not in _NC:
        _NC[keysc] = build_nc(keysc)
    return _NC[keysc]


def _shards(x, context_mask, gamma, wq, wkv, null_kv, q_scale, k_scale, wo,
            keysc):
    kcn = keysc // P
    x = np.asarray(x, np.float32)
    gamma = np.asarray(gamma, np.float32)
    wq_g = (np.asarray(wq, np.float32) * gamma[:, None]).astype(BF)
    wkv_g = np.asarray(wkv, np.float32) * gamma[:, None]
    wk_g = wkv_g[:, :D].astype(BF)
    wv_g = wkv_g[:, D:].astype(BF)
    wo = np.asarray(wo, np.float32)
    null_kv = np.asarray(null_kv, np.float32)
    cm = np.asarray(context_mask)
    qs = np.asarray(q_scale, np.float32)
    ks = np.asarray(k_scale, np.float32)
    qks = np.tile(qs * ks, 2).astype(np.float32)[:, None]  # [128,1]

    maps, perms = [], []
    for c in range(8):
        b, hh = c // 2, c % 2
        sl = slice(hh * IC, (hh + 1) * IC)
        heads = np.arange(HC) + hh * HC
        nk = null_kv[0][heads, 0, :]
        nv = null_kv[1][heads, 0, :]
        nullk = np.ascontiguousarray(
            nk.reshape(4, 2, 64).transpose(1, 2, 0).reshape(P, 4))
        nullv = np.zeros((1, HC * 65), np.float32)
        for h in range(HC):
            nullv[0, h * 65:h * 65 + 64] = nv[h]
            nullv[0, h * 65 + 64] = 1.0
        vis = np.flatnonzero(cm[b])
        perm = np.concatenate([vis, np.flatnonzero(~cm[b])])
        perms.append(perm)
        nvis = len(vis)
        bias = np.zeros((keysc,), np.float32)
        bias[nvis:] = NEG
        bias[keysc - 1] = 0.0        # null key always visible
        maskcol = np.ascontiguousarray(bias.reshape(kcn, P).T)
        xp = x[b][perm]
        maps.append({
            "xT": np.ascontiguousarray(xp.T).astype(BF),
            "wq": np.ascontiguousarray(wq_g[:, sl]),
            "wk": np.ascontiguousarray(wk_g[:, sl]),
            "wv": np.ascontiguousarray(wv_g[:, sl]),
            "wo": np.ascontiguousarray(wo[sl, :]).astype(BF),
            "nullk": nullk.astype(BF),
            "nullv": nullv.astype(BF),
            "maskcol": maskcol,
            "qks": qks,
        })
    return maps, perms


def kernel(x, context_mask, gamma, wq, wkv, null_kv, q_scale, k_scale, wo,
           _trace=False):
    cm = np.asarray(context_mask)
    max_vis = int(cm.sum(axis=1).max())
    # need max_vis visible slots plus the null key at slot keysc-1
    keysc = max(KEYSC, ((max_vis + 1 + P - 1) // P) * P)
    nc = _get_nc(keysc)
    maps, perms = _shards(x, context_mask, gamma, wq, wkv, null_kv,
                          q_scale, k_scale, wo, keysc)
    res = run_bass_kernel_spmd(nc, maps, core_ids=list(range(8)),
                               trace=_trace,
                               tmpdir="/tmp/bass_trace" if _trace else None)
    outs = [np.asarray(res.results[c]["out"], np.float32) for c in range(8)]
    full = np.empty((4, N, D), np.float32)
    for b in range(4):
        full[b, perms[2 * b], :] = outs[2 * b] + outs[2 * b + 1]
    if _trace:
        kernel.last_exec_time_ns = res.exec_time_ns
    return full


# revision 16
# speedup vs baseline: 1.4533x; 1.4533x over previous
"""Distributed attention kernel for 8 TRN2 NeuronCores (v3).

Sharding: core c -> (batch b = c//2, head-half hh = c%2).  Each core computes
LN(x_b) for all 2048 rows, q for its 8 heads, k/v over the first KEYSC-1
rows of a HOST-PERMUTED x (visible rows first, masked rows after; the output
is un-permuted on the host), l2norm cosine attention with the null k/v
appended at slot KEYSC-1, and a partial out @ wo[head-slice].  Host sums the
two partial outputs per batch.

v3 notes: the PE HAM clock gate re-throttles to 1.2 GHz after any ~3.4us PE
idle window, so every phase boundary gap matters.  The l2-norm stage shares
the projection-phase PSUM pool (so its matmuls interleave with the tail of
the projections), the softmax division runs per-m inside the attention loop,
and all 1/x go through the single-op reciprocal_approx_fast.
"""

import sys

sys.path.insert(0, "/opt/trn_rl_repo")

import numpy as np  # noqa: E402
import ml_dtypes  # noqa: E402

import concourse.bacc as bacc  # noqa: E402
import concourse.bass as bass  # noqa: E402
import concourse.tile as tile  # noqa: E402
from concourse import mybir  # noqa: E402
from concourse.bass_utils import run_bass_kernel_spmd  # noqa: E402

BF = ml_dtypes.bfloat16
F32 = mybir.dt.float32
BF16 = mybir.dt.bfloat16
AF = mybir.ActivationFunctionType
MUL = mybir.AluOpType.mult

P = 128
N = 2048          # query rows per batch
D = 1024          # model dim
HC = 8            # heads per core
IC = 512          # inner dim per core
NEG = -1.0e4
EPS_LN = 1e-5
EPS_L2 = 1e-12
SCALE = 8.0

KEYSC = 1152      # key slots: [0:nvis) visible, pads, null at KEYSC-1


def _chunks(total, step=512):
    return [(c, min(c + step, total)) for c in range(0, total, step)]


def build_nc(keysc=KEYSC):
    kcn = keysc // P
    nc = bacc.Bacc(None, target_bir_lowering=False)

    xT_d = nc.dram_tensor("xT", [D, N], BF16, kind="ExternalInput")
    wq_d = nc.dram_tensor("wq", [D, IC], BF16, kind="ExternalInput")
    wk_d = nc.dram_tensor("wk", [D, IC], BF16, kind="ExternalInput")
    wv_d = nc.dram_tensor("wv", [D, IC], BF16, kind="ExternalInput")
    wo_d = nc.dram_tensor("wo", [IC, D], BF16, kind="ExternalInput")
    nullk_d = nc.dram_tensor("nullk", [P, 4], BF16, kind="ExternalInput")
    nullv_d = nc.dram_tensor("nullv", [1, HC * 65], BF16, kind="ExternalInput")
    mask_d = nc.dram_tensor("maskcol", [P, kcn], F32, kind="ExternalInput")
    qks_d = nc.dram_tensor("qks", [P, 1], F32, kind="ExternalInput")
    out_d = nc.dram_tensor("out", [N, D], F32, kind="ExternalOutput")

    with tile.TileContext(nc) as tc:
        with (
            tc.tile_pool(name="consts", bufs=1) as cns,
            tc.tile_pool(name="qkv", bufs=1) as qkv,
            tc.tile_pool(name="wop", bufs=1) as wop,
            tc.tile_pool(name="rep", bufs=4) as repp,
            tc.tile_pool(name="dram", bufs=1, space="DRAM") as drp,
        ):
            qTh = [qkv.tile([P, 4, N], BF16, name=f"qTh{i}")
                   for i in range(2)]
            kT = qkv.tile([P, 4, keysc], BF16)
            v_sb = qkv.tile([P, kcn, HC * 65 + 63], BF16)
            oT = qkv.tile([P, 4, N], BF16)

            bnc_dr = drp.tile([2, N], BF16, name="bnc")
            den_dr = drp.tile([8, N], BF16, name="den")
            rec_dr = drp.tile([8, N], BF16, name="rec")

            # ---------- phase A+B: LN, projections, l2 norms ----------
            with (
                tc.tile_pool(name="xp", bufs=1) as xp,
                tc.tile_pool(name="wp", bufs=1) as wp,
                tc.tile_pool(name="asml", bufs=2) as sml,
                tc.tile_pool(name="ascr", bufs=2) as scr,
                tc.tile_pool(name="bsml", bufs=2) as bml,
                tc.tile_pool(name="lnps", bufs=2, space="PSUM") as lnps,
                tc.tile_pool(name="pjps", bufs=3, space="PSUM") as pjps,
            ):
                xT = xp.tile([P, 8, N], BF16)
                xre = xT_d.rearrange("(f p) r -> f p r", p=P)
                # first x chunk ahead of everything else in the DMA queues
                for f in range(8):
                    nc.sync.dma_start(out=xT[:, f, 0:512],
                                      in_=xre[f, :, 0:512])
                wq_sb = wp.tile([P, 8, IC], BF16, tag="wq")
                wk_sb = wp.tile([P, 8, IC], BF16, tag="wk")
                wv_sb = wp.tile([P, 8, IC], BF16, tag="wv")
                nc.sync.dma_start(
                    out=wq_sb, in_=wq_d.rearrange("(f p) j -> p f j", p=P))
                nc.sync.dma_start(
                    out=wk_sb, in_=wk_d.rearrange("(f p) j -> p f j", p=P))
                nc.sync.dma_start(
                    out=wv_sb, in_=wv_d.rearrange("(f p) j -> p f j", p=P))

                ones1b = cns.tile([P, 1], BF16)
                nc.vector.memset(ones1b, 1.0)
                blkdiag = cns.tile([P, 2], BF16)
                nc.vector.memset(blkdiag, 0.0)
                nc.vector.memset(blkdiag[0:64, 0:1], 1.0)
                nc.vector.memset(blkdiag[64:128, 1:2], 1.0)
                maskc = cns.tile([P, kcn], F32)
                nc.sync.dma_start(out=maskc, in_=mask_d[:, :])
                nullk_sb = cns.tile([P, 4], BF16)
                nc.sync.dma_start(out=nullk_sb, in_=nullk_d[:, :])
                qks_sb = cns.tile([P, 1], F32)
                nc.sync.dma_start(out=qks_sb, in_=qks_d[:, :])
                eps_col = cns.tile([P, 1], F32)
                nc.vector.memset(eps_col, EPS_LN)
                sc_col = cns.tile([P, 1], F32)
                nc.vector.memset(sc_col, 1.0 / float(D * D))
                eps2_col = cns.tile([P, 1], F32)
                nc.vector.memset(eps2_col, EPS_L2 * EPS_L2)
                wo_sb = wop.tile([P, 4, D], BF16)
                nc.sync.dma_start(
                    out=wo_sb, in_=wo_d.rearrange("(m p) j -> p m j", p=P))
                nc.vector.memset(
                    v_sb[:, :, 0:HC * 65].rearrange(
                        "p t (h c) -> p t h c", c=65)[:, :, :, 64:65],
                    1.0)
                nc.vector.memset(v_sb[:, :, HC * 65:], 0.0)
                nc.vector.memset(qTh[0][64:128, :, :], 0.0)
                nc.vector.memset(qTh[1][0:64, :, :], 0.0)

                for ci, (c0, c1) in enumerate(_chunks(N)):
                    w = c1 - c0
                    if ci > 0:
                        for f in range(8):
                            nc.sync.dma_start(out=xT[:, f, c0:c1],
                                              in_=xre[f, :, c0:c1])
                    sA = lnps.tile([1, 512], F32, tag="lA", name="sA")
                    sB = lnps.tile([1, 512], F32, tag="lB", name="sB")
                    for f in range(8):
                        xc = xT[:, f, c0:c1]
                        sq = scr.tile([P, 512], BF16, tag="sq", name="sq")
                        nc.vector.tensor_mul(sq[:, 0:w], xc, xc)
                        nc.tensor.matmul(sA[:, 0:w], ones1b, xc,
                                         start=(f == 0), stop=(f == 7))
                        nc.tensor.matmul(sB[:, 0:w], ones1b, sq[:, 0:w],
                                         start=(f == 0), stop=(f == 7))
                    # s = 1/sqrt(var+eps); ms = mean*s  (rows bf16)
                    sumr = sml.tile([1, 512], F32, tag="sumr", name="sumr")
                    nc.vector.tensor_copy(sumr[:, 0:w], sA[:, 0:w])
                    a1 = sml.tile([1, 512], F32, tag="a1", name="a1")
                    nc.vector.tensor_mul(a1[:, 0:w], sumr[:, 0:w],
                                         sumr[:, 0:w])
                    t1 = sml.tile([1, 512], F32, tag="t1", name="t1")
                    nc.vector.tensor_scalar(t1[:, 0:w], sB[:, 0:w],
                                            float(D), None, MUL)
                    nc.vector.tensor_sub(t1[:, 0:w], t1[:, 0:w], a1[:, 0:w])
                    sd = sml.tile([1, 512], F32, tag="sd", name="sd")
                    nc.scalar.activation(sd[:, 0:w], t1[:, 0:w], AF.Sqrt,
                                         scale=sc_col[0:1, :],
                                         bias=eps_col[0:1, :])
                    sf = sml.tile([1, 512], F32, tag="sf", name="sf")
                    nc.vector.reciprocal_approx_fast(sf[:, 0:w], sd[:, 0:w])
                    row_s = sml.tile([1, 512], BF16, tag="rows", name="row_s")
                    nc.vector.tensor_copy(row_s[:, 0:w], sf[:, 0:w])
                    nc.vector.tensor_mul(a1[:, 0:w], sumr[:, 0:w],
                                         sf[:, 0:w])
                    row_m = sml.tile([1, 512], BF16, tag="rowm", name="row_m")
                    nc.vector.tensor_scalar(row_m[:, 0:w], a1[:, 0:w],
                                            1.0 / float(D), None, MUL)
                    nc.sync.dma_start(out=bnc_dr[0:1, c0:c1],
                                      in_=row_s[:, 0:w])
                    nc.sync.dma_start(out=bnc_dr[1:2, c0:c1],
                                      in_=row_m[:, 0:w])
                    rep_s = repp.tile([P, 512], BF16, tag="rep", name="rep_s")
                    rep_m = repp.tile([P, 512], BF16, tag="rep", name="rep_m")
                    for (i, r) in ((0, rep_s), (1, rep_m)):
                        src = bnc_dr[i, c0:c1]
                        nc.sync.dma_start(
                            out=r[:, 0:w],
                            in_=bass.AP(tensor=src.tensor, offset=src.offset,
                                        ap=[[0, P]] + src.ap))
                    for f in range(8):
                        nc.vector.tensor_mul(xT[:, f, c0:c1], xT[:, f, c0:c1],
                                             rep_s[:, 0:w])
                        nc.vector.tensor_sub(xT[:, f, c0:c1], xT[:, f, c0:c1],
                                             rep_m[:, 0:w])
                    # q projection for this chunk
                    for m in range(4):
                        qp = pjps.tile([P, 512], F32, tag="pj", name="qp")
                        for f in range(8):
                            nc.tensor.matmul(
                                qp[:, 0:w], wq_sb[:, f, m * P:(m + 1) * P],
                                xT[:, f, c0:c1],
                                start=(f == 0), stop=(f == 7))
                        nc.scalar.copy(qTh[0][0:64, m, c0:c1],
                                       qp[0:64, 0:w])
                        nc.vector.tensor_copy(qTh[1][64:128, m, c0:c1],
                                              qp[64:128, 0:w])
                    # k projection (only columns < keysc)
                    k1 = min(c1, keysc)
                    if c0 < keysc:
                        kw = k1 - c0
                        for m in range(4):
                            kp = pjps.tile([P, 512], F32, tag="pj", name="kp")
                            for f in range(8):
                                nc.tensor.matmul(
                                    kp[:, 0:kw],
                                    wk_sb[:, f, m * P:(m + 1) * P],
                                    xT[:, f, c0:k1],
                                    start=(f == 0), stop=(f == 7))
                            nc.scalar.copy(kT[:, m, c0:k1], kp[:, 0:kw])
                        # v projection: 128-row tiles within this chunk
                        for rt in range(c0 // P, k1 // P):
                            vp = pjps.tile([P, 512], F32, tag="pj", name="vp")
                            for f in range(8):
                                nc.tensor.matmul(
                                    vp, xT[:, f, rt * P:(rt + 1) * P],
                                    wv_sb[:, f, :],
                                    start=(f == 0), stop=(f == 7))
                            nc.scalar.copy(
                                v_sb[:, rt, 0:HC * 65].rearrange(
                                    "p (h c) -> p h c", c=65)[:, :, 0:64],
                                vp.rearrange("p (h c) -> p h c", c=64))
                    if ci == 2:
                        # k/v fully projected: null k/v overwrite (keysc-1)
                        nc.sync.dma_start(
                            out=kT[:, :, keysc - 1:keysc],
                            in_=nullk_sb.rearrange("p (m o) -> p m o", o=1))
                        nc.sync.dma_start(
                            out=v_sb[127:128, kcn - 1, 0:HC * 65],
                            in_=nullv_d[:, :])
                # l2 norms per m (k done after chunk 2, q after chunk 3)
                for m in range(4):
                    sqk = scr.tile([P, keysc], BF16, tag="sqq", bufs=2,
                                   name="sqk")
                    nc.vector.tensor_mul(sqk, kT[:, m, :], kT[:, m, :])
                    nstk = bml.tile([2, keysc], F32, tag="l2a", name="nstk")
                    for (d0, d1) in _chunks(keysc):
                        t2 = pjps.tile([2, 512], F32, tag="pj", name="t2k")
                        nc.tensor.matmul(t2[:, 0:d1 - d0], blkdiag,
                                         sqk[:, d0:d1], start=True, stop=True)
                        nc.vector.tensor_copy(nstk[:, d0:d1],
                                              t2[:, 0:d1 - d0])
                    sdk = bml.tile([2, keysc], F32, tag="l2a", name="sdk")
                    nc.scalar.activation(sdk, nstk, AF.Sqrt,
                                         bias=eps2_col[0:2, :])
                    nc.vector.reciprocal_approx_fast(nstk, sdk)
                    nbk = bml.tile([2, keysc], BF16, tag="l2b", name="nbk")
                    nc.vector.tensor_scalar(nbk, nstk, SCALE, None, MUL)
                    bk = drp.tile([2, keysc], BF16, tag="bk", bufs=4,
                                  name=f"bk{m}")
                    nc.sync.dma_start(out=bk, in_=nbk)
                    repk = repp.tile([P, keysc], BF16, tag="rpq", bufs=2,
                                     name="repk")
                    for h2 in range(2):
                        src_ = bk[h2, :]
                        nc.sync.dma_start(
                            out=repk[64 * h2:64 * (h2 + 1), :],
                            in_=bass.AP(tensor=src_.tensor,
                                        offset=src_.offset,
                                        ap=[[0, 64]] + src_.ap))
                    nc.vector.tensor_mul(kT[:, m, :], kT[:, m, :], repk)
                    sqq = scr.tile([P, N], BF16, tag="sqq", bufs=2,
                                   name="sqq")
                    nc.vector.tensor_mul(sqq[0:64, :], qTh[0][0:64, m, :],
                                         qTh[0][0:64, m, :])
                    nc.vector.tensor_mul(sqq[64:128, :], qTh[1][64:128, m, :],
                                         qTh[1][64:128, m, :])
                    nst = bml.tile([2, N], F32, tag="l2a", name="nstq")
                    for (d0, d1) in _chunks(N):
                        t2 = pjps.tile([2, 512], F32, tag="pj", name="t2")
                        nc.tensor.matmul(t2[:, 0:d1 - d0], blkdiag,
                                         sqq[:, d0:d1], start=True, stop=True)
                        nc.vector.tensor_copy(nst[:, d0:d1],
                                              t2[:, 0:d1 - d0])
                    sd2 = bml.tile([2, N], F32, tag="l2a", name="sd2")
                    nc.scalar.activation(sd2, nst, AF.Sqrt,
                                         bias=eps2_col[0:2, :])
                    nc.vector.reciprocal_approx_fast(nst, sd2)
                    nbf = bml.tile([2, N], BF16, tag="l2b", name="nbfq")
                    nc.vector.tensor_copy(nbf, nst)
                    bq = drp.tile([2, N], BF16, tag="bq", bufs=4,
                                  name=f"bq{m}")
                    nc.sync.dma_start(out=bq, in_=nbf)
                    repq = repp.tile([P, N], BF16, tag="rpq", bufs=2,
                                     name="repq")
                    for h2 in range(2):
                        src = bq[h2, :]
                        nc.sync.dma_start(
                            out=repq[64 * h2:64 * (h2 + 1), :],
                            in_=bass.AP(tensor=src.tensor, offset=src.offset,
                                        ap=[[0, 64]] + src.ap))
                    nc.vector.tensor_scalar(repq, repq, qks_sb[:, 0:1],
                                            None, MUL)
                    nc.vector.tensor_mul(qTh[0][0:64, m, :],
                                         qTh[0][0:64, m, :], repq[0:64, :])
                    nc.vector.tensor_mul(qTh[1][64:128, m, :],
                                         qTh[1][64:128, m, :],
                                         repq[64:128, :])

            # ---------- phase C: attention + per-m softmax division --------
            with (
                tc.tile_pool(name="accp", bufs=2, space="PSUM") as accp,
                tc.tile_pool(name="simp", bufs=2, space="PSUM") as simp,
                tc.tile_pool(name="expp", bufs=3) as expp,
                tc.tile_pool(name="omp", bufs=3) as omp,
                tc.tile_pool(name="dsml", bufs=2) as dsml,
            ):
                for m in range(4):
                    for rc in range(2):
                        ops = [accp.tile([P, 1024], F32, tag="acc",
                                         name=f"av{i}") for i in range(2)]
                        for kc in range(kcn):
                            for h2 in range(2):
                                sim = simp.tile([P, 1024], F32, tag="sim",
                                                name="sim")
                                for nh in range(2):
                                    r0 = rc * 1024 + nh * 512
                                    nc.tensor.matmul(
                                        sim[:, nh * 512:(nh + 1) * 512],
                                        kT[:, m, kc * P:(kc + 1) * P],
                                        qTh[h2][:, m, r0:r0 + 512],
                                        start=True, stop=True)
                                e = expp.tile([P, 1024], BF16, tag="e",
                                              name="e")
                                nc.scalar.activation(
                                    e, sim, AF.Exp,
                                    bias=maskc[:, kc:kc + 1])
                                for nh in range(2):
                                    nc.tensor.matmul(
                                        ops[h2][:, nh * 512:(nh + 1) * 512],
                                        v_sb[:, kc,
                                             (2 * m + h2) * 65:
                                             (2 * m + h2) * 65 + 128],
                                        e[:, nh * 512:(nh + 1) * 512],
                                        start=(kc == 0), stop=(kc == kcn - 1))
                        for h2 in range(2):
                            om = omp.tile([65, 1024], BF16, tag="om",
                                          name="om")
                            with nc.allow_low_precision("bf16 numer/denom"):
                                nc.vector.tensor_copy(om, ops[h2][0:65, :])
                            nc.sync.dma_start(
                                out=oT[64 * h2:64 * (h2 + 1), m,
                                       rc * 1024:(rc + 1) * 1024],
                                in_=om[0:64, :])
                            nc.sync.dma_start(
                                out=den_dr[2 * m + h2:2 * m + h2 + 1,
                                           rc * 1024:(rc + 1) * 1024],
                                in_=om[64:65, :])
                    # divide this m's numerators while m+1 attends
                    dsb = dsml.tile([2, N], BF16, tag="dsb", name="dsb")
                    nc.sync.dma_start(out=dsb, in_=den_dr[2 * m:2 * m + 2, :])
                    dff = dsml.tile([2, N], F32, tag="dff", name="dff")
                    nc.vector.tensor_copy(dff, dsb)
                    drf = dsml.tile([2, N], F32, tag="dff", name="drf")
                    nc.vector.reciprocal_approx_fast(drf, dff)
                    drb = dsml.tile([2, N], BF16, tag="dsb", name="drb")
                    with nc.allow_low_precision("bf16 recip"):
                        nc.vector.tensor_copy(drb, drf)
                    nc.sync.dma_start(out=rec_dr[2 * m:2 * m + 2, :], in_=drb)
                    repd = repp.tile([P, N], BF16, tag="rpq", bufs=2,
                                     name="repd")
                    for h2 in range(2):
                        src = rec_dr[2 * m + h2, :]
                        nc.sync.dma_start(
                            out=repd[64 * h2:64 * (h2 + 1), :],
                            in_=bass.AP(tensor=src.tensor, offset=src.offset,
                                        ap=[[0, 64]] + src.ap))
                    nc.vector.tensor_mul(oT[:, m, :], oT[:, m, :], repd)

            # ---------- phase D: output projection ----------
            with (
                tc.tile_pool(name="dps", bufs=4, space="PSUM") as dps,
                tc.tile_pool(name="dscr", bufs=3) as scr2,
            ):
                for rt in range(16):
                    for n2 in range(2):
                        op = dps.tile([P, 512], F32, tag="op", name="op")
                        for m in range(4):
                            nc.tensor.matmul(
                                op, oT[:, m, rt * P:(rt + 1) * P],
                                wo_sb[:, m, n2 * 512:(n2 + 1) * 512],
                                start=(m == 0), stop=(m == 3))
                        sg = scr2.tile([P, 512], F32, tag="sg", name="sg")
                        if n2 == 0:
                            nc.scalar.copy(sg, op)
                        else:
                            nc.vector.tensor_copy(sg, op)
                        nc.sync.dma_start(
                            out=out_d[rt * P:(rt + 1) * P,
                                      n2 * 512:(n2 + 1) * 512],
                            in_=sg)

    nc.finalize()
    return nc


_NC = {}


def _get_nc(keysc=KEYSC):
    if keysc not in _NC:
        _NC[keysc] = build_nc(keysc)
    return _NC[keysc]


def _shards(x, context_mask, gamma, wq, wkv, null_kv, q_scale, k_scale, wo,
            keysc):
    kcn = keysc // P
    x = np.asarray(x, np.float32)
    gamma = np.asarray(gamma, np.float32)
    wq_g = (np.asarray(wq, np.float32) * gamma[:, None]).astype(BF)
    wkv_g = np.asarray(wkv, np.float32) * gamma[:, None]
    wk_g = wkv_g[:, :D].astype(BF)
    wv_g = wkv_g[:, D:].astype(BF)
    wo = np.asarray(wo, np.float32)
    null_kv = np.asarray(null_kv, np.float32)
    cm = np.asarray(context_mask)
    qs = np.asarray(q_scale, np.float32)
    ks = np.asarray(k_scale, np.float32)
    qks = np.tile(qs * ks, 2).astype(np.float32)[:, None]  # [128,1]

    maps, perms = [], []
    for c in range(8):
        b, hh = c // 2, c % 2
        sl = slice(hh * IC, (hh + 1) * IC)
        heads = np.arange(HC) + hh * HC
        nk = null_kv[0][heads, 0, :]
        nv = null_kv[1][heads, 0, :]
        nullk = np.ascontiguousarray(
            nk.reshape(4, 2, 64).transpose(1, 2, 0).reshape(P, 4))
        nullv = np.zeros((1, HC * 65), np.float32)
        for h in range(HC):
            nullv[0, h * 65:h * 65 + 64] = nv[h]
            nullv[0, h * 65 + 64] = 1.0
        vis = np.flatnonzero(cm[b])
        perm = np.concatenate([vis, np.flatnonzero(~cm[b])])
        perms.append(perm)
        nvis = len(vis)
        bias = np.zeros((keysc,), np.float32)
        bias[nvis:] = NEG
        bias[keysc - 1] = 0.0        # null key always visible
        maskcol = np.ascontiguousarray(bias.reshape(kcn, P).T)
        xp = x[b][perm]
        maps.append({
            "xT": np.ascontiguousarray(xp.T).astype(BF),
            "wq": np.ascontiguousarray(wq_g[:, sl]),
            "wk": np.ascontiguousarray(wk_g[:, sl]),
            "wv": np.ascontiguousarray(wv_g[:, sl]),
            "wo": np.ascontiguousarray(wo[sl, :]).astype(BF),
            "nullk": nullk.astype(BF),
            "nullv": nullv.astype(BF),
            "maskcol": maskcol,
            "qks": qks,
        })
    return maps, perms


def kernel(x, context_mask, gamma, wq, wkv, null_kv, q_scale, k_scale, wo,
           _trace=False):
    cm = np.asarray(context_mask)
    max_vis = int(cm.sum(axis=1).max())
    # need max_vis visible slots plus the null key at slot keysc-1
    keysc = max(KEYSC, ((max_vis + 1 + P - 1) // P) * P)
    nc = _get_nc(keysc)
    maps, perms = _shards(x, context_mask, gamma, wq, wkv, null_kv,
                          q_scale, k_scale, wo, keysc)
    res = run_bass_kernel_spmd(nc, maps, core_ids=list(range(8)),
                               trace=_trace,
                               tmpdir="/tmp/bass_trace" if _trace else None)
    outs = [np.asarray(res.results[c]["out"], np.float32) for c in range(8)]
    full = np.empty((4, N, D), np.float32)
    for b in range(4):
        full[b, perms[2 * b], :] = outs[2 * b] + outs[2 * b + 1]
    if _trace:
        kernel.last_exec_time_ns = res.exec_time_ns
    return full
